# revision 47
# baseline (speedup 1.0000x reference)
"""AttributeBBoxHead forward on 8 trn2 NeuronCores.

Computes 5 fused linear heads: out = x.reshape(8192, 12544) @ W_cat + b_cat
with W_cat = concat of the 5 head weights (12544, 168), data-parallel over
the RoI axis (1024 RoIs per core), W replicated.

Per-core bass/tile kernel (~195-208 us HW, rel err ~1.8e-4 vs fp32
reference; DMA-bound: 59.8 MB/core at ~356 GB/s = 168 us floor):
  - x shard streamed HBM->SBUF as [128p, 8rb, 896k] slabs (3.5KB runs,
    ~356 GB/s), W chunks interleaved on the same HWDGE ring
  - each 128x128 block of x transposed on the PE in float32r transpose-mode
    (1.5 cyc/row), staged through two half-k-tile PSUM tiles so the
    PSUM -> SBUF copies (scalar+vector, rounding to float32r) start early
  - out^T[attr, roi] accumulated in PSUM over 98 k-tiles, W stationary in
    float32r (fp32-replication mode: 1 cyc/row at moving dim >= 256, vs 4
    for plain fp32), software-pipelined one k-tile behind the transposes
  - eviction via ACT/DVE with bias add, DMA out as [168, 1024]
Host concatenates per-core out^T.T and splits the 5 heads.
"""

import numpy as np

N_ROIS = 8192
D = 12544  # 256 * 7 * 7
N_CORES = 8
R = N_ROIS // N_CORES  # 1024 rois per core
KI = 128
KO = D // KI  # 98 k-tiles
RB = R // 128  # 8 roi blocks per core
A_TOTAL = 168  # 32 + 124 + 3 + 7 + 2
A1 = 128  # first attr chunk (partitions of out^T)
A2 = A_TOTAL - A1  # 40
KT_PER_SLAB = 7  # 98 = 14 slabs x 7 k-tiles; 896 k per slab = 3.5KB runs
N_SLABS = KO // KT_PER_SLAB
SLAB_K = KT_PER_SLAB * KI  # 896

HEAD_SPLITS = [("cls", 32), ("reg", 124), ("face", 3), ("colour", 7), ("motion", 2)]

_CACHE = {}


def _emit_kernel(tc, x, w, b, out_t):
    """Emit the per-core kernel. x: [R, D], w: [128, KO, A_TOTAL] view,
    b: [A_TOTAL], out_t: [A_TOTAL, R] (all DRAM APs)."""
    from contextlib import ExitStack

    import concourse.bass as bass  # noqa: F401
    import concourse.mybir as mybir
    from concourse.masks import make_identity

    nc = tc.nc
    f32 = mybir.dt.float32

    ctx = ExitStack()
    singles = ctx.enter_context(tc.tile_pool(name="singles", bufs=1))
    slab_pool = ctx.enter_context(tc.tile_pool(name="slabs", bufs=3))
    xt_pool = ctx.enter_context(tc.tile_pool(name="xt", bufs=3))
    out_pool = ctx.enter_context(tc.tile_pool(name="outs", bufs=1))
    xp_pool = ctx.enter_context(tc.tile_pool(name="xp", bufs=4, space="PSUM"))
    acc_pool = ctx.enter_context(tc.tile_pool(name="acc", bufs=1, space="PSUM"))

    f32r = mybir.dt.float32r

    ident = singles.tile([128, 128], f32)
    make_identity(nc, ident)
    # f32r copy of the identity for f32r transpose-mode (values 0/1 round
    # exactly); produced by a cast so the fp32r-producer check is satisfied
    ident_r = singles.tile([128, 128], f32r)
    nc.vector.tensor_copy(ident_r, ident)

    # W resident in SBUF: [ki, ko, a]. float32r (PE fp32-replication mode);
    # host pre-rounds W to bf16-hi+lo representable values. Loaded in 7
    # chunks interleaved with the slab stream (emitted in the slab loop).
    w_sb = singles.tile([KI, KO, A_TOTAL], f32r)
    W_CHUNK = 14

    # bias as per-partition scalars for the out^T orientation
    # bias scatter (4B per partition, ~130 tiny descriptors) goes on the
    # SWDGE ring: it is only needed at eviction time and must not block
    # the slab stream on the sync HWDGE ring
    b_col = b.rearrange("(p one) -> p one", one=1)  # [168, 1]
    b1 = singles.tile([A1, 1], f32)
    b2 = singles.tile([A2, 1], f32)
    nc.gpsimd.dma_start(out=b1, in_=b_col[0:A1, :])
    nc.gpsimd.dma_start(out=b2, in_=b_col[A1:A_TOTAL, :])

    # x as [p, rb, k]
    x3 = x.rearrange("(rb p) k -> p rb k", p=128)

    # out^T accumulators, live across the whole k loop (4 PSUM banks).
    # The 40-attr chunk is col-tiled: rois 0:512 accumulate at array cols
    # 0:40 into acc2, rois 512:1024 at cols 64:104 into acc3 — the two
    # moving streams run concurrently through separate XBUS groups.
    acc = [
        acc_pool.tile([A1, 512], f32, name="acc0"),
        acc_pool.tile([A1, 512], f32, name="acc1"),
        acc_pool.tile([A2, 512], f32, name="acc2"),
        acc_pool.tile([A2, 512], f32, name="acc3"),
    ]

    # Software pipeline: compute matmuls for k-tile kt-1 are emitted after the
    # transposes for kt, so the PE transposes the next tile while DVE/ACT
    # stage the current one (hides the PSUM->SBUF copy latency).
    xts = {}

    def emit_computes(kt):
        # out^T += W_tile.T @ xT  (W stationary, shared lhsT adjacent).
        # float32r: PE fp32-replication mode, ~4x faster than plain fp32
        # at moving dim >= 256.
        first = kt == 0
        last = kt == KO - 1
        xt = xts.pop(kt)
        w1 = w_sb[:, kt, 0:A1]
        w2 = w_sb[:, kt, A1:A_TOTAL]
        nc.tensor.matmul(acc[0], lhsT=w1, rhs=xt[:, 0:512], start=first, stop=last)
        nc.tensor.matmul(acc[1], lhsT=w1, rhs=xt[:, 512:R], start=first, stop=last)
        nc.tensor.matmul(acc[2], lhsT=w2, rhs=xt[:, 0:512], start=first, stop=last)
        nc.tensor.matmul(acc[3], lhsT=w2, rhs=xt[:, 512:R], start=first, stop=last)

    # variable slab widths: small first slab so the PE starts early
    slab_kts = [2, 7] + [7] * (N_SLABS - 2) + [5]
    assert sum(slab_kts) == KO
    kt0 = 0
    for slab_i, skt in enumerate(slab_kts):
        xslab = slab_pool.tile([128, RB, SLAB_K], f32r, tag="xslab", name="xslab")
        nc.sync.dma_start(
            out=xslab[:, :, 0 : skt * KI],
            in_=x3[:, :, kt0 * KI : (kt0 + skt) * KI],
        )
        # interleave W chunks into the slab stream on the same ring: chunk i
        # lands ~14 k-tiles before its first consumer
        if slab_i < KO // W_CHUNK:
            wc = slab_i * W_CHUNK
            nc.sync.dma_start(
                out=w_sb[:, wc : wc + W_CHUNK, :], in_=w[:, wc : wc + W_CHUNK, :]
            )
        for j in range(skt):
            kt = kt0 + j

            # transpose 8 [128,128] blocks of x via PE transpose-mode
            # (f32r transpose-mode streams at 1.5 cyc/row vs 2 for fp32);
            # two half-k-tile PSUM tiles so staging starts after 4 blocks
            xt = xt_pool.tile([128, R], f32r, tag="xt", name="xt")
            xpa = xp_pool.tile([128, 512], f32r, tag="xp", name="xpa")
            for rb in range(4):
                nc.tensor.transpose(
                    xpa[:, rb * 128 : (rb + 1) * 128],
                    xslab[:, rb, j * KI : (j + 1) * KI],
                    ident_r,
                )
            nc.scalar.copy(xt[:, 0:512], xpa)
            xpb = xp_pool.tile([128, 512], f32r, tag="xp", name="xpb")
            for rb in range(4, RB):
                nc.tensor.transpose(
                    xpb[:, (rb - 4) * 128 : (rb - 3) * 128],
                    xslab[:, rb, j * KI : (j + 1) * KI],
                    ident_r,
                )
            nc.vector.tensor_copy(xt[:, 512:R], xpb)
            xts[kt] = xt

            if kt >= 1:
                emit_computes(kt - 1)
        kt0 += skt
    emit_computes(KO - 1)

    # evict with bias add (Identity activation, per-partition bias AP);
    # split across ACT and DVE. acc3's chunk lives at partitions 64:104.
    out_sb1 = out_pool.tile([A1, R], f32)
    out_sb2 = out_pool.tile([A2, R], f32)
    b2_bcast = bass.AP(tensor=b2.tensor, offset=b2.offset, ap=[b2.ap[0], [0, 512]])
    nc.scalar.add(out_sb1[:, 0:512], acc[0], add=b1)
    nc.vector.tensor_add(out_sb2[:, 0:512], acc[2], b2_bcast)
    nc.scalar.add(out_sb1[:, 512:R], acc[1], add=b1)
    nc.vector.tensor_add(out_sb2[:, 512:R], acc[3], b2_bcast)
    nc.sync.dma_start(out=out_t[0:A1, :], in_=out_sb1)
    nc.sync.dma_start(out=out_t[A1:A_TOTAL, :], in_=out_sb2)
    ctx.close()


def build_bass():
    """Build (and cache) the compiled bass module."""
    if "nc" in _CACHE:
        return _CACHE["nc"]
    import concourse.mybir as mybir
    import concourse.tile as tile
    from concourse import bacc

    f32 = mybir.dt.float32
    nc = bacc.Bacc("TRN2", debug=False, num_devices=N_CORES)
    x_d = nc.dram_tensor(
        "x_shard", [R, D], mybir.dt.float32r, kind="ExternalInput"
    ).ap()
    # W pre-shuffled on host to the SBUF layout [ki, ko, a] so the load is
    # one contiguous 64KB run per partition
    w_d = nc.dram_tensor(
        "w_kia", [KI, KO, A_TOTAL], mybir.dt.float32r, kind="ExternalInput"
    ).ap()
    b_d = nc.dram_tensor("b_cat", [A_TOTAL], f32, kind="ExternalInput").ap()
    o_d = nc.dram_tensor("out_t", [A_TOTAL, R], f32, kind="ExternalOutput").ap()

    w_view = w_d

    with tile.TileContext(nc) as tc:
        _emit_kernel(tc, x_d, w_view, b_d, o_d)

    nc.compile()
    _CACHE["nc"] = nc
    return nc


def make_in_maps(x, W_cat, b_cat):
    xf = np.ascontiguousarray(x.reshape(N_ROIS, D).astype(np.float32))
    # [D, A] -> [ki, ko, a] with ko*KI + ki = k
    w_kia = np.ascontiguousarray(
        W_cat.reshape(KO, KI, A_TOTAL).transpose(1, 0, 2)
    )
    return [
        {
            "x_shard": xf[i * R : (i + 1) * R],
            "w_kia": w_kia,
            "b_cat": b_cat,
        }
        for i in range(N_CORES)
    ]


def assemble_output(results):
    """results: list of per-core dicts with out_t [168, 1024]."""
    full = np.concatenate([np.asarray(r["out_t"]).T for r in results], axis=0)
    outs = []
    ofs = 0
    for _, dim in HEAD_SPLITS:
        outs.append(np.ascontiguousarray(full[:, ofs : ofs + dim]))
        ofs += dim
    return tuple(outs)


def round_to_bf16_pair(a):
    """Round fp32 values to the nearest bf16-hi + bf16-lo representable value,
    so the PE's fp32r hi/lo split reproduces them exactly."""
    import ml_dtypes

    a = np.asarray(a, np.float32)
    hi = a.astype(ml_dtypes.bfloat16).astype(np.float32)
    lo = (a - hi).astype(ml_dtypes.bfloat16).astype(np.float32)
    return hi + lo


def kernel(x, W_cls, b_cls, W_reg, b_reg, W_face, b_face,
           W_colour, b_colour, W_motion, b_motion):
    from concourse.bass_utils import run_bass_kernel_spmd

    W_cat = np.ascontiguousarray(
        round_to_bf16_pair(
            np.concatenate(
                [np.asarray(w, np.float32) for w in (W_cls, W_reg, W_face, W_colour, W_motion)],
                axis=1,
            )
        )
    )
    b_cat = np.ascontiguousarray(
        np.concatenate(
            [np.asarray(b, np.float32) for b in (b_cls, b_reg, b_face, b_colour, b_motion)]
        )
    )
    nc = build_bass()
    in_maps = make_in_maps(np.asarray(x), W_cat, b_cat)
    res = run_bass_kernel_spmd(nc, in_maps, core_ids=list(range(N_CORES)))
    return assemble_output(res.results)


# revision 48
# speedup vs baseline: 1.0721x; 1.0721x over previous
"""AttributeBBoxHead forward on 8 trn2 NeuronCores.

Computes 5 fused linear heads: out = x.reshape(8192, 12544) @ W_cat + b_cat
with W_cat = concat of the 5 head weights (12544, 168), data-parallel over
the RoI axis (1024 RoIs per core), W replicated.

Per-core bass/tile kernel (~195-208 us HW, rel err ~1.8e-4 vs fp32
reference; DMA-bound: 59.8 MB/core at ~356 GB/s = 168 us floor):
  - x shard streamed HBM->SBUF as [128p, 8rb, 896k] slabs (3.5KB runs,
    ~356 GB/s), W chunks interleaved on the same HWDGE ring
  - each 128x128 block of x transposed on the PE in float32r transpose-mode
    (1.5 cyc/row), staged through two half-k-tile PSUM tiles so the
    PSUM -> SBUF copies (scalar+vector, rounding to float32r) start early
  - out^T[attr, roi] accumulated in PSUM over 98 k-tiles, W stationary in
    float32r (fp32-replication mode: 1 cyc/row at moving dim >= 256, vs 4
    for plain fp32), software-pipelined one k-tile behind the transposes
  - eviction via ACT/DVE with bias add, DMA out as [168, 1024]
Host concatenates per-core out^T.T and splits the 5 heads.
"""

import numpy as np

N_ROIS = 8192
D = 12544  # 256 * 7 * 7
N_CORES = 8
R = N_ROIS // N_CORES  # 1024 rois per core
KI = 128
KO = D // KI  # 98 k-tiles
RB = R // 128  # 8 roi blocks per core
A_TOTAL = 168  # 32 + 124 + 3 + 7 + 2
A1 = 128  # first attr chunk (partitions of out^T)
A2 = A_TOTAL - A1  # 40
KT_PER_SLAB = 7  # 98 = 14 slabs x 7 k-tiles; 896 k per slab = 3.5KB runs
N_SLABS = KO // KT_PER_SLAB
SLAB_K = KT_PER_SLAB * KI  # 896

HEAD_SPLITS = [("cls", 32), ("reg", 124), ("face", 3), ("colour", 7), ("motion", 2)]

_CACHE = {}


def _emit_kernel(tc, x, w, b, out_t):
    """Emit the per-core kernel. x: [R, D], w: [128, KO, A_TOTAL] view,
    b: [A_TOTAL], out_t: [A_TOTAL, R] (all DRAM APs)."""
    from contextlib import ExitStack

    import concourse.bass as bass  # noqa: F401
    import concourse.mybir as mybir
    from concourse.masks import make_identity

    nc = tc.nc
    f32 = mybir.dt.float32

    ctx = ExitStack()
    singles = ctx.enter_context(tc.tile_pool(name="singles", bufs=1))
    slab_pool = ctx.enter_context(tc.tile_pool(name="slabs", bufs=3))
    xt_pool = ctx.enter_context(tc.tile_pool(name="xt", bufs=4))
    out_pool = ctx.enter_context(tc.tile_pool(name="outs", bufs=1))
    xp_pool = ctx.enter_context(tc.tile_pool(name="xp", bufs=4, space="PSUM"))
    acc_pool = ctx.enter_context(tc.tile_pool(name="acc", bufs=1, space="PSUM"))

    f32r = mybir.dt.float32r

    ident = singles.tile([128, 128], f32)
    make_identity(nc, ident)
    # f32r copy of the identity for f32r transpose-mode (values 0/1 round
    # exactly); produced by a cast so the fp32r-producer check is satisfied
    ident_r = singles.tile([128, 128], f32r)
    nc.vector.tensor_copy(ident_r, ident)

    # W resident in SBUF: [ki, ko, a]. float32r (PE fp32-replication mode);
    # host pre-rounds W to bf16-hi+lo representable values. Loaded in 7
    # chunks interleaved with the slab stream (emitted in the slab loop).
    w_sb = singles.tile([KI, KO, A_TOTAL], f32r)
    W_CHUNK = 14

    # bias as per-partition scalars for the out^T orientation
    # bias scatter (4B per partition, ~130 tiny descriptors) goes on the
    # SWDGE ring: it is only needed at eviction time and must not block
    # the slab stream on the sync HWDGE ring
    b_col = b.rearrange("(p one) -> p one", one=1)  # [168, 1]
    b1 = singles.tile([A1, 1], f32)
    b2 = singles.tile([A2, 1], f32)
    nc.gpsimd.dma_start(out=b1, in_=b_col[0:A1, :])
    nc.gpsimd.dma_start(out=b2, in_=b_col[A1:A_TOTAL, :])

    # x as [p, rb, k]
    x3 = x.rearrange("(rb p) k -> p rb k", p=128)

    # out^T accumulators, live across the whole k loop (4 PSUM banks).
    # The 40-attr chunk is col-tiled: rois 0:512 accumulate at array cols
    # 0:40 into acc2, rois 512:1024 at cols 64:104 into acc3 — the two
    # moving streams run concurrently through separate XBUS groups.
    acc = [
        acc_pool.tile([A1, 512], f32, name="acc0"),
        acc_pool.tile([A1, 512], f32, name="acc1"),
        acc_pool.tile([A2, 512], f32, name="acc2"),
        acc_pool.tile([A2, 512], f32, name="acc3"),
    ]

    # Software pipeline: compute matmuls for k-tile kt-1 are emitted after the
    # transposes for kt, so the PE transposes the next tile while DVE/ACT
    # stage the current one (hides the PSUM->SBUF copy latency).
    xts = {}

    def emit_computes(kt):
        # out^T += W_tile.T @ xT  (W stationary, shared lhsT adjacent).
        # float32r: PE fp32-replication mode, ~4x faster than plain fp32
        # at moving dim >= 256.
        first = kt == 0
        last = kt == KO - 1
        xt = xts.pop(kt)
        w1 = w_sb[:, kt, 0:A1]
        w2 = w_sb[:, kt, A1:A_TOTAL]
        nc.tensor.matmul(acc[0], lhsT=w1, rhs=xt[:, 0:512], start=first, stop=last)
        nc.tensor.matmul(acc[1], lhsT=w1, rhs=xt[:, 512:R], start=first, stop=last)
        nc.tensor.matmul(acc[2], lhsT=w2, rhs=xt[:, 0:512], start=first, stop=last)
        nc.tensor.matmul(acc[3], lhsT=w2, rhs=xt[:, 512:R], start=first, stop=last)

    # variable slab widths: small first slab so the PE starts early
    slab_kts = [2, 7] + [7] * (N_SLABS - 2) + [5]
    assert sum(slab_kts) == KO
    kt0 = 0
    for slab_i, skt in enumerate(slab_kts):
        xslab = slab_pool.tile([128, RB, SLAB_K], f32r, tag="xslab", name="xslab")
        nc.sync.dma_start(
            out=xslab[:, :, 0 : skt * KI],
            in_=x3[:, :, kt0 * KI : (kt0 + skt) * KI],
        )
        # interleave W chunks into the slab stream on the same ring; the
        # first chunk is tiny so slab 1 isn't delayed behind a 1.2MB W load
        W_SPANS = [(0, 4), (4, 18), (18, 32), (32, 46), (46, 60), (60, 74),
                   (74, 88), (88, 98)]
        if slab_i < len(W_SPANS):
            wa, wb = W_SPANS[slab_i]
            nc.sync.dma_start(out=w_sb[:, wa:wb, :], in_=w[:, wa:wb, :])
        for j in range(skt):
            kt = kt0 + j

            # transpose 8 [128,128] blocks of x via PE transpose-mode
            # (f32r transpose-mode streams at 1.5 cyc/row vs 2 for fp32);
            # two half-k-tile PSUM tiles so staging starts after 4 blocks
            xt = xt_pool.tile([128, R], f32r, tag="xt", name="xt")
            xpa = xp_pool.tile([128, 512], f32r, tag="xp", name="xpa")
            for rb in range(4):
                nc.tensor.transpose(
                    xpa[:, rb * 128 : (rb + 1) * 128],
                    xslab[:, rb, j * KI : (j + 1) * KI],
                    ident_r,
                )
            nc.scalar.copy(xt[:, 0:512], xpa)
            xpb = xp_pool.tile([128, 512], f32r, tag="xp", name="xpb")
            for rb in range(4, RB):
                nc.tensor.transpose(
                    xpb[:, (rb - 4) * 128 : (rb - 3) * 128],
                    xslab[:, rb, j * KI : (j + 1) * KI],
                    ident_r,
                )
            nc.vector.tensor_copy(xt[:, 512:R], xpb)
            xts[kt] = xt

            if kt >= 1:
                emit_computes(kt - 1)
        kt0 += skt
    emit_computes(KO - 1)

    # evict with bias add (Identity activation, per-partition bias AP);
    # split across ACT and DVE. acc3's chunk lives at partitions 64:104.
    out_sb1 = out_pool.tile([A1, R], f32)
    out_sb2 = out_pool.tile([A2, R], f32)
    b2_bcast = bass.AP(tensor=b2.tensor, offset=b2.offset, ap=[b2.ap[0], [0, 512]])
    nc.scalar.add(out_sb1[:, 0:512], acc[0], add=b1)
    nc.vector.tensor_add(out_sb2[:, 0:512], acc[2], b2_bcast)
    nc.scalar.add(out_sb1[:, 512:R], acc[1], add=b1)
    nc.vector.tensor_add(out_sb2[:, 512:R], acc[3], b2_bcast)
    nc.sync.dma_start(out=out_t[0:A1, :], in_=out_sb1)
    nc.sync.dma_start(out=out_t[A1:A_TOTAL, :], in_=out_sb2)
    ctx.close()


def build_bass():
    """Build (and cache) the compiled bass module."""
    if "nc" in _CACHE:
        return _CACHE["nc"]
    import concourse.mybir as mybir
    import concourse.tile as tile
    from concourse import bacc

    f32 = mybir.dt.float32
    nc = bacc.Bacc("TRN2", debug=False, num_devices=N_CORES)
    x_d = nc.dram_tensor(
        "x_shard", [R, D], mybir.dt.float32r, kind="ExternalInput"
    ).ap()
    # W pre-shuffled on host to the SBUF layout [ki, ko, a] so the load is
    # one contiguous 64KB run per partition
    w_d = nc.dram_tensor(
        "w_kia", [KI, KO, A_TOTAL], mybir.dt.float32r, kind="ExternalInput"
    ).ap()
    b_d = nc.dram_tensor("b_cat", [A_TOTAL], f32, kind="ExternalInput").ap()
    o_d = nc.dram_tensor("out_t", [A_TOTAL, R], f32, kind="ExternalOutput").ap()

    w_view = w_d

    with tile.TileContext(nc) as tc:
        _emit_kernel(tc, x_d, w_view, b_d, o_d)

    nc.compile()
    _CACHE["nc"] = nc
    return nc


def make_in_maps(x, W_cat, b_cat):
    xf = np.ascontiguousarray(x.reshape(N_ROIS, D).astype(np.float32))
    # [D, A] -> [ki, ko, a] with ko*KI + ki = k
    w_kia = np.ascontiguousarray(
        W_cat.reshape(KO, KI, A_TOTAL).transpose(1, 0, 2)
    )
    return [
        {
            "x_shard": xf[i * R : (i + 1) * R],
            "w_kia": w_kia,
            "b_cat": b_cat,
        }
        for i in range(N_CORES)
    ]


def assemble_output(results):
    """results: list of per-core dicts with out_t [168, 1024]."""
    full = np.concatenate([np.asarray(r["out_t"]).T for r in results], axis=0)
    outs = []
    ofs = 0
    for _, dim in HEAD_SPLITS:
        outs.append(np.ascontiguousarray(full[:, ofs : ofs + dim]))
        ofs += dim
    return tuple(outs)


def round_to_bf16_pair(a):
    """Round fp32 values to the nearest bf16-hi + bf16-lo representable value,
    so the PE's fp32r hi/lo split reproduces them exactly."""
    import ml_dtypes

    a = np.asarray(a, np.float32)
    hi = a.astype(ml_dtypes.bfloat16).astype(np.float32)
    lo = (a - hi).astype(ml_dtypes.bfloat16).astype(np.float32)
    return hi + lo


def kernel(x, W_cls, b_cls, W_reg, b_reg, W_face, b_face,
           W_colour, b_colour, W_motion, b_motion):
    from concourse.bass_utils import run_bass_kernel_spmd

    W_cat = np.ascontiguousarray(
        round_to_bf16_pair(
            np.concatenate(
                [np.asarray(w, np.float32) for w in (W_cls, W_reg, W_face, W_colour, W_motion)],
                axis=1,
            )
        )
    )
    b_cat = np.ascontiguousarray(
        np.concatenate(
            [np.asarray(b, np.float32) for b in (b_cls, b_reg, b_face, b_colour, b_motion)]
        )
    )
    nc = build_bass()
    in_maps = make_in_maps(np.asarray(x), W_cat, b_cat)
    res = run_bass_kernel_spmd(nc, in_maps, core_ids=list(range(N_CORES)))
    return assemble_output(res.results)
